# revision 49
# baseline (speedup 1.0000x reference)
"""Trainium2 Bass kernel for complex-valued multi-head attention with key masking.

Problem (hardcoded shapes): B=4, Nq=Nk=1024, R=256, NH=8, DK=DV=64.
  Q,K,V complex [B,N,R] (given as _real/_imag f32 pairs), complex weights
  WQ/WK/WV [512,256], WO [256,512], boolean key mask [B,Nk].
  out = complex MHA(Q,K,V) with softmax over |scores| restricted to valid keys.

Sharding: 8 cores = (batch b in 0..3) x (head-group hg in 0..1, 4 heads each).
Each core computes its batch's attention for its 4 heads plus the partial
output projection; the host sums the two head-group partials per batch.

Layout: everything transposed (channels on partitions, sequence on free dim).
Complex arithmetic is folded into matmuls by stacking real/imag along the
128-partition contraction dim.  Scores are computed TRANSPOSED (S^T[k,q]:
lhsT = K-projection block, rhs = Q-projection), so the softmaxed weights are
already in the [k,q] layout the attention matmul needs.  The imaginary part
uses Kb = swap-halves(Ka) with one half negated (sign is irrelevant since
only si^2 enters |s|^2); Kb is 640 cols vs a swapped Q's 1024.

Softmax: |s|^2 via two drain ops (real-part square on DVE or ACT-Square,
imag-part sq-accumulate on DVE -- the DVE may read only ONE PSUM operand
per instruction), then a SINGLE activation pass through a custom pwp table
(see _gen_act_tables) that computes e = exp(min(sqrt(u), 8)) in the `sqrt`
slot: no separate sqrt pass, no exp pass, no table swaps, and padded keys
(u == +0.0 exactly) map to e = 0 through the table's fzero path, so no
mask-bias tensor exists at all.  The sq-accumulate of tile k is emitted
one kb-slot after its scores + first square, and the table pass two slots
after, so neither in-order queue ever head-of-line blocks on the other.
Denominator: e-tiles are pre-summed over key blocks on the otherwise-idle
GPSIMD (bf16 adds), then a SINGLE ones-vector matmul pair per head
(replaces 10 PE matmuls per head with 2), reciprocal on DVE, broadcast
across partitions via a small DRAM-bounce DMA, applied as the PSUM->SBUF
drain of the attention accumulator one head later.

Pipeline (head h): PE runs scores(h) against a 3-deep PSUM ring (1.5 key
blocks of runway over the drains), attn(h-1) shifted two kb slots late,
den(h-1) once at the end; projections for h0/h1 run up front; h2/h3 and
the V*Wv blocks are spread into the early heads' slots.  Outputs are bf16,
summed across the two head-group cores on the host.
"""

import os

import numpy as np
import ml_dtypes

B, NQ, NK, R = 4, 1024, 1024, 256
NH, DK, DV = 8, 64, 64
NCORES = 8
NHL = 4          # heads per core
F32MIN_PAD = 640  # minimum padded key count (keys padded to a multiple of 128)

_BF16 = ml_dtypes.bfloat16

# ----------------------------------------------------------------------------
# custom activation table: the `sqrt` slot of the `sqrt_and_others` set is
# rewritten to compute f(u) = exp(min(sqrt(u), 8)) so the whole
# softmax-numerator of the |score| softmax is ONE activation pass (no sqrt
# pass, no exp pass, no table swaps).  The pwp format is exponent-bucketed
# cubics: 32-byte entries [c0,c1,c2,c3,x0,0,0,0] (f32) evaluating
# c0+c1*dx+c2*dx^2+c3*dx^3 with dx = x-x0; sub-bucket count per binade is
# (next exponent's start index - this one's).  fzero_result for the sqrt
# slot is +0.0, which makes padded keys (u == +0 exactly) produce e = 0 --
# exactly the masking the softmax needs, with no bias tensor at all.
# Max |score|/8 over this problem's inputs is ~1.87, so the cap at 8 gives
# 4x margin; the table reproduces f to ~6e-6 relative.
# ----------------------------------------------------------------------------
_EXP_CAP = 8.0
_TABLE_DIR = [None]


def _gen_act_tables():
    if _TABLE_DIR[0] is not None:
        return _TABLE_DIR[0]
    import json
    import shutil
    import tempfile
    from neuronxcc.driver.Job import Job
    from neuronxcc.driver.jobs.support.FindActInfo import findActInfoFile
    src_dir = os.path.dirname(findActInfoFile(Job.getPackageDir(), "gen3"))
    dst_dir = tempfile.mkdtemp(prefix="cmha_pwp_")
    for fn in os.listdir(src_dir):
        s = os.path.join(src_dir, fn)
        if os.path.isfile(s):
            shutil.copy(s, os.path.join(dst_dir, fn))

    def f(u):
        return np.exp(np.minimum(np.sqrt(np.maximum(u, 0.0)), _EXP_CAP))

    pj = json.load(open(os.path.join(src_dir, "sqrt_and_others.json")))
    bkt = np.fromfile(os.path.join(src_dir, pj["bkt_bin"]),
                      dtype=np.uint8).reshape(-1, 32).copy()
    em = pj["func_exp_to_bkt_start_idx"]["sqrt"]
    meta = next(m for m in pj["profile_meta_data"]
                if m["func_name"].startswith("sqrt"))
    exps = sorted(int(k) for k in em.keys())
    starts = {int(k): v[0] for k, v in em.items()}
    first_special = meta["pos_small_signal_pwl_control"]
    for i, e in enumerate(exps):
        nxt = starts[exps[i + 1]] if i + 1 < len(exps) else first_special
        n = nxt - starts[e]
        base = 2.0 ** e
        for j in range(n):
            lo, hi = base * (1.0 + j / n), base * (1.0 + (j + 1) / n)
            x0 = 0.5 * (lo + hi)
            xs = np.linspace(lo, hi, 64, dtype=np.float64)
            ys = f(xs)
            w = 1.0 / np.maximum(ys, 1e-300)   # relative-error weighting
            V = np.stack([np.ones_like(xs), xs - x0, (xs - x0) ** 2,
                          (xs - x0) ** 3], 1)
            c, *_ = np.linalg.lstsq(V * w[:, None], ys * w, rcond=None)
            ent = np.zeros(8, np.float32)
            ent[0:4] = c.astype(np.float32)
            ent[4] = np.float32(x0)
            bkt[starts[e] + j] = ent.view(np.uint8)
    sm = np.zeros(8, np.float32); sm[0] = 1.0          # u -> 0+: f = 1
    lg = np.zeros(8, np.float32); lg[0] = np.float32(np.exp(_EXP_CAP))
    bkt[meta["pos_small_signal_pwl_control"]] = sm.view(np.uint8)
    bkt[meta["pos_large_signal_pwl_control"]] = lg.view(np.uint8)
    bkt.tofile(os.path.join(dst_dir, pj["bkt_bin"]))

    path = os.path.join(dst_dir, "act_info.json")
    os.environ["BASS_ACT_ROOT_JSON_PATH"] = path
    _TABLE_DIR[0] = path
    return path


# ----------------------------------------------------------------------------
# custom DVE ops (registered at import into concourse's op table)
# ----------------------------------------------------------------------------
_OPS = {}


def _register_custom_ops():
    if _OPS:
        return
    import concourse.dve_ops as dom
    from concourse.dve_ops import DveOp
    from concourse.dve_spec import Spec, Src0, Src1, C0, C1, sq, lower, _has_src1
    from concourse.dve_uop import DveOpSpec

    def make(name, spec):
        if name in dom._SUB_OPCODE_FOR_NAME:
            _OPS[name] = next(o for o in dom.OPS if o.name == name)
            return
        row = dom._CUSTOM_DVE_ROW_BASE + len(dom.OPS)
        assert row < 0x20, "custom DVE row overflow"
        shas = {}
        for ver in ("v3", "v4"):
            tmp = DveOpSpec(name=name, opcode=row, uops=lower(spec, ver=ver),
                            rd1_en=_has_src1(spec))
            shas[ver] = tmp.sha(ver)
        op = DveOp(name, spec, subdim=False, uops_sha=shas)
        dom.OPS.append(op)
        dom._SUB_OPCODE_FOR_NAME[name] = row
        dom.CUSTOM_DVE_SPECS[name] = spec
        _OPS[name] = op

    # t = (in0*s0)^2          (drains+squares one score tile from PSUM;
    # the DVE can read at most ONE non-scalar input from PSUM, so |s|^2
    # takes two ops: sq the real part, then sq-accumulate the imag part)
    make("CMHA_SQSC", Spec(
        body=sq(Src0 * C0),
        reference=lambda in0, in1, s0, s1, imm2: (in0.astype(np.float32) * s0) ** 2,
    ))
    # u = (in0*s0)^2 + in1 + s1   (second square, accumulate |s|^2; s1 is a
    # tiny epsilon so sqrt never sees an exact 0)
    make("CMHA_SQADD", Spec(
        body=sq(Src0 * C0) + Src1 + C1,
        reference=lambda in0, in1, s0, s1, imm2: (in0.astype(np.float32) * s0) ** 2
        + in1.astype(np.float32) + s1,
    ))


# ----------------------------------------------------------------------------
# device program
# ----------------------------------------------------------------------------
_BUILD_CACHE = {}


def _build(nkp):
    """Build + compile the SPMD device program for padded key count nkp."""
    if nkp in _BUILD_CACHE:
        return _BUILD_CACHE[nkp]
    _gen_act_tables()
    _register_custom_ops()
    import concourse.bass as bass
    import concourse.bacc as bacc
    import concourse.mybir as mybir
    import concourse.tile as tile
    from contextlib import ExitStack

    F32 = mybir.dt.float32
    BF16 = mybir.dt.bfloat16
    AF = mybir.ActivationFunctionType
    assert nkp % 128 == 0
    KB = nkp // 128
    kchunks = [(o, min(512, nkp - o)) for o in range(0, nkp, 512)]

    nc = bacc.Bacc("TRN2", target_bir_lowering=False, debug=False,
                   num_devices=NCORES)

    qt = nc.dram_tensor("qt", [128, 4 * NQ], BF16, kind="ExternalInput").ap()
    kt = nc.dram_tensor("kt", [128, 4 * nkp], BF16, kind="ExternalInput").ap()
    vt = nc.dram_tensor("vt", [128, 4 * nkp], BF16, kind="ExternalInput").ap()
    wq = nc.dram_tensor("wq", [128, NHL * 512], BF16, kind="ExternalInput").ap()
    wk = nc.dram_tensor("wk", [128, NHL * 512], BF16, kind="ExternalInput").ap()
    wv = nc.dram_tensor("wv", [128, 4 * 512], BF16, kind="ExternalInput").ap()
    wo = nc.dram_tensor("wo", [128, NHL * 512], BF16, kind="ExternalInput").ap()
    outr = nc.dram_tensor("outr", [256, NQ], BF16, kind="ExternalOutput").ap()
    outi = nc.dram_tensor("outi", [256, NQ], BF16, kind="ExternalOutput").ap()

    sqsc = _OPS["CMHA_SQSC"]
    sqadd = _OPS["CMHA_SQADD"]

    with tile.TileContext(nc) as tc, ExitStack() as ctx:
        const = ctx.enter_context(tc.tile_pool(name="const", bufs=1))
        # PSUM: scores/den/WO ring (3x [128,1024] = 6 banks; depth 3 gives
        # the PE 1.5 key-blocks of runway over the DVE/ACT drains) +
        # attention accumulator (2 banks) = 8 banks.
        psp = ctx.enter_context(tc.tile_pool(name="psp", bufs=3, space="PSUM"))
        acc = ctx.enter_context(tc.tile_pool(name="acc", bufs=1, space="PSUM"))
        prj = ctx.enter_context(tc.tile_pool(name="prj", bufs=1))
        smv = ctx.enter_context(tc.tile_pool(name="smv", bufs=8))
        esb = ctx.enter_context(tc.tile_pool(name="esb", bufs=11))
        esm = ctx.enter_context(tc.tile_pool(name="esm", bufs=2))
        nrm = ctx.enter_context(tc.tile_pool(name="nrm", bufs=2))
        drp = ctx.enter_context(tc.tile_pool(name="drp", bufs=2, space="DRAM"))
        outp = ctx.enter_context(tc.tile_pool(name="outp", bufs=4))

        # ---- input loads: priority-ordered column pieces round-robined over
        # the three DMA-capable queues.  Head 0's operands (wq/wk h0, qt, kt)
        # land first so the first projection starts as early as possible. ----
        _eng = [nc.sync, nc.scalar, nc.gpsimd]
        _rr = [0]

        def mk(shape, dtype, tag):
            return const.tile(shape, dtype, tag=tag, name=tag)

        def piece(t, src, a, b):
            _eng[_rr[0] % 3].dma_start(t[:, a:b], src[:, a:b])
            _rr[0] += 1

        wq_sb = mk([128, NHL * 512], BF16, "wq")
        qt_sb = mk([128, 4 * NQ], BF16, "qt")
        wk_sb = mk([128, NHL * 512], BF16, "wk")
        kt_sb = mk([128, 4 * nkp], BF16, "kt")
        wv_sb = mk([128, 4 * 512], BF16, "wv")
        vt_sb = mk([128, 4 * nkp], BF16, "vt")
        wo_sb = mk([128, NHL * 512], BF16, "wo")

        # Few, large pieces: DMA descriptors are per partition row, and
        # rows under 2KB run the rings at half efficiency.  The two big
        # streams (qt 1MB / kt+weights) ride different hardware-DGE queues
        # in parallel; the V path and wo ride the gpsimd software-DGE since
        # they are not needed until the head-0 attention blocks.
        def pc(eng, t, src, a, b):
            eng.dma_start(t[:, a:b], src[:, a:b])

        pc(nc.scalar, wq_sb, wq, 0, 512)             # head-0 Q weights
        for c in range(4):                           # qt full chunks (2KB rows)
            pc(nc.sync, qt_sb, qt, c * NQ, (c + 1) * NQ)
        pc(nc.scalar, wk_sb, wk, 0, 512)             # head-0 K weights
        pc(nc.scalar, kt_sb, kt, 0, 4 * nkp)         # kt in one piece
        pc(nc.scalar, wq_sb, wq, 512, 1024)          # head-1 weights
        pc(nc.scalar, wk_sb, wk, 512, 1024)
        pc(nc.gpsimd, wv_sb, wv, 0, 2048)            # V path (head-0 VK blocks)
        pc(nc.gpsimd, vt_sb, vt, 0, 4 * nkp)
        pc(nc.scalar, wk_sb, wk, 1024, 2048)         # remaining heads
        pc(nc.scalar, wq_sb, wq, 1024, 2048)
        pc(nc.gpsimd, wo_sb, wo, 0, 2048)

        VK = const.tile([128, 512 * KB], BF16, tag="vk", name="VK")
        ATT = [const.tile([128, NQ], BF16, tag=f"att{h}", name=f"ATT{h}")
               for h in range(NHL)]
        ones = const.tile([128, 1], BF16, tag="ones", name="ones")
        nc.vector.memset(ones[:], 1.0)
        # pin the sqrt_and_others table set immediately (the custom f lives
        # in its sqrt slot; Square/Copy are resident there too, so this is
        # the only table load in the whole program)
        scr = const.tile([1, 1], BF16, tag="scr", name="scr")
        nc.scalar.activation(scr[:], ones[0:1, 0:1], AF.Sqrt)

        def mm(out_ap, lhsT, rhs, start=True, stop=True):
            nc.tensor.matmul(out_ap, lhsT, rhs, start=start, stop=stop)

        # ---- projections ---------------------------------------------------
        qa_sb = [None] * NHL
        ka_sb = [None] * NHL
        kb_sb = [None] * NHL

        def emit_proj_q(h, drain_eng=None, between=None):
            # `between` (front only) emits other PE work between the two
            # q-half groups, filling the wait for the qc1 qt DMA pieces
            qa_ps = psp.tile([128, 1024], F32, tag="ps", name="qa_ps")
            for qc in range(2):
                for c in range(4):
                    mm(qa_ps[:, qc * 512:(qc + 1) * 512],
                       wq_sb[:, h * 512 + c * 128:h * 512 + (c + 1) * 128],
                       qt_sb[:, c * NQ + qc * 512:c * NQ + (qc + 1) * 512],
                       c == 0, c == 3)
                if qc == 0 and between is not None:
                    between()
            qa = prj.tile([128, NQ], BF16, tag=f"qa{h}", name="qa")
            # front heads drain on the idle ACT; spread heads drain on the
            # DVE so the sqrt/exp pipeline is not delayed
            if drain_eng == "vector":
                nc.vector.tensor_copy(qa[:], qa_ps[:])
            else:
                nc.scalar.copy(qa[:], qa_ps[:])
            qa_sb[h] = qa

        def emit_proj_k(h):
            ka_ps = psp.tile([128, 1024], F32, tag="ps", name="ka_ps")
            for c in range(4):
                for (o, w_) in kchunks:
                    mm(ka_ps[:, o:o + w_],
                       wk_sb[:, h * 512 + c * 128:h * 512 + (c + 1) * 128],
                       kt_sb[:, c * nkp + o:c * nkp + o + w_], c == 0, c == 3)
            ka = prj.tile([128, nkp], BF16, tag=f"ka{h}", name="ka")
            nc.scalar.copy(ka[:], ka_ps[:, 0:nkp])
            ka_sb[h] = ka
            # Kb = [Ka_imag-half; -Ka_real-half]: swap the halves via
            # SBUF->SBUF DMA on two queues; the negate (sign irrelevant for
            # |s|^2, either half works) runs on the DVE.
            kb_t = prj.tile([128, nkp], BF16, tag=f"kb{h}", name="kb_t")
            nc.sync.dma_start(kb_t[0:64, :], ka[64:128, :])
            nc.gpsimd.dma_start(kb_t[64:128, :], ka[0:64, :])
            kb_sb[h] = kb_t

        def emit_kb_neg(h):
            nc.vector.tensor_scalar_mul(kb_sb[h][64:128, :],
                                        kb_sb[h][64:128, :], -1.0)

        def emit_vk_block(kb):
            ko = kb * 128
            ps = psp.tile([128, 512], F32, tag="ps", name="vk_ps")
            for c in range(4):
                mm(ps[:, 0:512], vt_sb[:, c * nkp + ko:c * nkp + ko + 128],
                   wv_sb[:, c * 512:(c + 1) * 512], c == 0, c == 3)
            nc.scalar.copy(VK[:, kb * 512:(kb + 1) * 512], ps[:, 0:512])

        # ---- per-head pipeline ---------------------------------------------
        e_tiles = [None] * NHL   # softmax numerators per head
        att_ps_l = [None] * NHL
        den_ps_l = [None] * NHL
        esum_l = [None] * NHL
        rdb_l = [None] * NHL

        t_tiles = {}
        si_tiles = {}

        def emit_scores_block(h, kb):
            # phase 1 of the softmax chain: scores + the real-part square.
            # The imag-part sq-accumulate and the table pass are emitted one
            # slot later (emit_softmax_tail) so the in-order ACT/DVE queues
            # never head-of-line block on each other's previous tile.
            ko = kb * 128
            sr = psp.tile([128, 1024], F32, tag="ps", name="sr")
            for qc in range(2):
                mm(sr[:, qc * 512:(qc + 1) * 512],
                   ka_sb[h][:, ko:ko + 128],
                   qa_sb[h][:, qc * 512:(qc + 1) * 512])
            si = psp.tile([128, 1024], F32, tag="ps", name="si")
            for qc in range(2):
                mm(si[:, qc * 512:(qc + 1) * 512],
                   kb_sb[h][:, ko:ko + 128],
                   qa_sb[h][:, qc * 512:(qc + 1) * 512])
            t = smv.tile([128, NQ], BF16, tag="t", name="t")
            if kb >= 2:
                # balance: some real-part squares per head run on the ACT
                # engine (Square is resident in the pinned table set)
                nc.scalar.activation(t[:], sr[:], AF.Square, scale=0.125)
            else:
                nc.vector._custom_dve(sqsc, out=t[:], in0=sr[:], s0=0.125)
            t_tiles[(h, kb)] = t
            si_tiles[(h, kb)] = si

        u_tiles = {}

        def emit_sqadd(h, kb):
            u = smv.tile([128, NQ], BF16, tag="u", name="u")
            # s1=0 keeps padded keys at u == +0.0 exactly, which the custom
            # table's fzero path maps to e = 0 (the key masking, for free)
            nc.vector._custom_dve(sqadd, out=u[:], in0=si_tiles[(h, kb)][:],
                                  in1=t_tiles[(h, kb)][:], s0=0.125, s1=0.0)
            u_tiles[(h, kb)] = u

        def emit_f(h, kb):
            e = esb.tile([128, NQ], BF16, tag="e", name="e")
            nc.scalar.activation(e[:], u_tiles[(h, kb)][:],
                                 AF.Sqrt)   # = exp(min(sqrt,8))
            e_tiles[h].append(e)

        def emit_attn_block(h, kb):
            for qc in range(2):
                mm(att_ps_l[h][:, qc * 512:(qc + 1) * 512],
                   VK[:, kb * 512 + h * 128: kb * 512 + (h + 1) * 128],
                   e_tiles[h][kb][:, qc * 512:(qc + 1) * 512],
                   start=(kb == 0), stop=(kb == KB - 1))

        def emit_esum_add(h, k):
            # running sum of the e-tiles on the otherwise-idle GPSIMD (all
            # SBUF bf16, InstTensorTensor from the standard Pool library):
            # k=1 seeds e0+e1, k>=2 accumulates
            s = esm.tile([128, NQ], BF16, tag="es", name="esum")
            if k == 1:
                nc.gpsimd.tensor_add(s[:], e_tiles[h][0][:], e_tiles[h][1][:])
            else:
                nc.gpsimd.tensor_add(s[:], esum_l[h][:], e_tiles[h][k][:])
            esum_l[h] = s

        def emit_den(h):
            den = psp.tile([128, 1024], F32, tag="ps", name="den_ps")
            mm(den[0:1, 0:512], ones[:, 0:1], esum_l[h][:, 0:512])
            mm(den[0:1, 512:1024], ones[:, 0:1], esum_l[h][:, 512:1024])
            den_ps_l[h] = den

        def emit_norm_recip(h, fast=False):
            from concourse.dve_ops import (RECIPROCAL_APPROX_FAST,
                                           RECIP_APPROX_FAST_CONSTS as RC)
            rden = nrm.tile([1, NQ], BF16, tag="rden", name="rden")
            rdd = drp.tile([1, NQ], BF16, tag="rdd", name="rdd")
            rdb = nrm.tile([128, NQ], BF16, tag="rdb", name="rdb")
            if fast:
                # tail path: pipeline per q-half and split the broadcast
                # across partition halves on two queues to cut the latency
                # between the denominator matmul and the final mult
                for qc in range(2):
                    s = slice(qc * 512, (qc + 1) * 512)
                    nc.vector._custom_dve(
                        RECIPROCAL_APPROX_FAST, out=rden[:, s],
                        in0=den_ps_l[h][0:1, s],
                        s0=RC["s0"], s1=RC["s1"], imm2=RC["imm2"])
                    nc.sync.dma_start(rdd[:, s], rden[:, s])
                    nc.sync.dma_start(rdb[0:64, s],
                                      rdd[:, s].to_broadcast((64, 512)))
                    nc.scalar.dma_start(rdb[64:128, s],
                                        rdd[:, s].to_broadcast((64, 512)))
            else:
                nc.vector._custom_dve(
                    RECIPROCAL_APPROX_FAST, out=rden[:],
                    in0=den_ps_l[h][0:1, :],
                    s0=RC["s0"], s1=RC["s1"], imm2=RC["imm2"])
                nc.sync.dma_start(rdd[:], rden[:])
                nc.sync.dma_start(rdb[:], rdd[:].to_broadcast((128, NQ)))
            rdb_l[h] = rdb

        def emit_norm_mult(h, fast=False):
            if fast:
                for qc in range(2):
                    s = slice(qc * 512, (qc + 1) * 512)
                    nc.vector.tensor_mul(ATT[h][:, s], att_ps_l[h][:, s],
                                         rdb_l[h][:, s])
            else:
                nc.vector.tensor_mul(ATT[h][:], att_ps_l[h][:], rdb_l[h][:])

        # ---- front: projections for heads 0 and 1 --------------------------
        emit_proj_q(0, between=lambda: emit_proj_k(0))
        emit_kb_neg(0)
        emit_proj_q(1)
        emit_proj_k(1)
        emit_kb_neg(1)

        # spread work items for later heads: (head, kb) -> list of thunks
        spread = {}

        def put(h, kb, fn):
            spread.setdefault((h, kb), []).append(fn)

        # VK blocks: 3 in head 0, 2 early in head 1
        for kb in range(KB):
            if kb < 3:
                put(0, kb + 1, lambda kb=kb: emit_vk_block(kb))
            else:
                put(1, kb - 3, lambda kb=kb: emit_vk_block(kb))
        # projections for heads 2/3 spread across heads 0/1
        put(0, 2, lambda: emit_proj_q(2))
        put(0, 4, lambda: emit_proj_k(2))
        put(0, 4, lambda: emit_kb_neg(2))
        put(1, 2, lambda: emit_proj_q(3))
        put(1, 4, lambda: emit_proj_k(3))
        put(1, 4, lambda: emit_kb_neg(3))

        # ---- head loop ------------------------------------------------------
        for h in range(NHL):
            e_tiles[h] = []
            if h >= 1:
                att_ps_l[h - 1] = acc.tile([128, 1024], F32, tag="attp",
                                           name="att_ps")
            for kb in range(KB):
                emit_scores_block(h, kb)
                if kb >= 2:
                    emit_f(h, kb - 2)
                if kb >= 1:
                    emit_sqadd(h, kb - 1)
                for fn in spread.get((h, kb), []):
                    fn()
                if h >= 1 and kb >= 2:
                    emit_attn_block(h - 1, kb - 2)
                    emit_esum_add(h - 1, kb - 1)
                if h >= 2 and kb == 1:
                    emit_norm_mult(h - 2)
            emit_sqadd(h, KB - 1)
            emit_f(h, KB - 2)
            emit_f(h, KB - 1)
            if h >= 1:
                emit_attn_block(h - 1, KB - 2)
                emit_attn_block(h - 1, KB - 1)
                emit_esum_add(h - 1, KB - 1)
                emit_den(h - 1)
                emit_norm_recip(h - 1)

        # ---- tail: last head's attention + output projection.  The last
        # head's denominator accumulates on the PE per key block (2 small
        # matmuls each) instead of the esum adds: the serial add chain would
        # sit directly on the tail's critical path. ----------------------
        h = NHL - 1
        att_ps_l[h] = acc.tile([128, 1024], F32, tag="attp", name="att_ps")
        den_t = psp.tile([128, 1024], F32, tag="ps", name="den_tail")
        for kb in range(KB):
            emit_attn_block(h, kb)
            for qc in range(2):
                mm(den_t[0:1, qc * 512:(qc + 1) * 512], ones[:, 0:1],
                   e_tiles[h][kb][:, qc * 512:(qc + 1) * 512],
                   start=(kb == 0), stop=(kb == KB - 1))
        den_ps_l[h] = den_t
        emit_norm_mult(h - 1)
        emit_norm_recip(h, fast=True)

        # output projection (accumulate over heads).  Pair 1 (outr) uses the
        # scores ring; pair 2 (outi) reuses the attention-accumulator slots,
        # whose release (h2/h3 normalization) is exactly the data dependency
        # of their h2/h3 matmuls.  Within each pair all h=0..2 matmuls are
        # emitted before any h=3 matmul, so the PE chews through them while
        # head 3 is still normalizing (strict in-order queue).
        groups = [(ri, blk) for ri in range(2) for blk in range(2)]
        pt = [psp.tile([128, 1024], F32, tag="ps", name="wo_ps"),
              psp.tile([128, 1024], F32, tag="ps", name="wo_ps2"),
              acc.tile([128, 1024], F32, tag="attp", name="wo_ps3"),
              psp.tile([128, 1024], F32, tag="ps", name="wo_ps4")]

        def emit_wo_mms(gis, hs):
            for h_ in hs:
                for gi in gis:
                    ri, blk = groups[gi]
                    lh = wo_sb[:, h_ * 512 + ri * 256 + blk * 128:
                               h_ * 512 + ri * 256 + (blk + 1) * 128]
                    for qc in range(2):
                        nc.tensor.matmul(
                            pt[gi][:, qc * 512:(qc + 1) * 512], lh,
                            ATT[h_][:, qc * 512:(qc + 1) * 512],
                            start=(h_ == 0), stop=(h_ == NHL - 1))

        emit_wo_mms([0, 1], range(NHL - 1))
        emit_wo_mms([3], range(NHL - 1))
        emit_norm_mult(h, fast=True)
        emit_wo_mms([0, 1], [NHL - 1])
        emit_wo_mms([3], [NHL - 1])
        emit_wo_mms([2], range(NHL))
        out_eng = [nc.sync, nc.scalar]
        oq = [0]
        for gi, (ri, blk) in enumerate(groups):
            osb = outp.tile([128, 1024], BF16, tag="osb", name="osb")
            # alternate drain engines so the four output drains pipeline
            if gi % 2 == 0:
                nc.scalar.copy(osb[:], pt[gi][:])
            else:
                nc.vector.tensor_copy(osb[:], pt[gi][:])
            dst = outr if ri == 0 else outi
            # one full-width DMA per tile: 2KB per partition row keeps the
            # descriptors at full efficiency (1KB halves ran at ~110GB/s)
            out_eng[oq[0] % 2].dma_start(
                dst[blk * 128:(blk + 1) * 128, :], osb[:])
            oq[0] += 1

    nc.compile()
    _BUILD_CACHE[nkp] = nc
    return nc


# ----------------------------------------------------------------------------
# host-side prep / gather
# ----------------------------------------------------------------------------
def _ctile(x):
    """[4*128, N] -> [128, 4*N] device layout (chunk-major columns)."""
    n = x.shape[1]
    return x.reshape(4, 128, n).transpose(1, 0, 2).reshape(128, 4 * n)


def _prep_inputs(Q_real, Q_imag, K_real, K_imag, V_real, V_imag,
                 WQ_r, WQ_i, WK_r, WK_i, WV_r, WV_i, WO_r, WO_i, mask):
    f32 = np.float32
    mask = np.asarray(mask).astype(bool)
    cnts = mask.sum(1)
    valid = mask.any(1)
    nkp = int(max(F32MIN_PAD, ((int(cnts.max()) + 127) // 128) * 128)) if valid.any() else F32MIN_PAD
    KB = nkp // 128

    # weight stacks (shared across cores up to head-group slicing)
    A_q = np.concatenate([WQ_r.T, -WQ_i.T], 0).astype(f32)   # [512, 512]
    B_q = np.concatenate([WQ_i.T, WQ_r.T], 0).astype(f32)
    A_k = np.concatenate([WK_r.T, -WK_i.T], 0).astype(f32)
    B_k = np.concatenate([WK_i.T, WK_r.T], 0).astype(f32)
    A_v = np.concatenate([WV_r.T, -WV_i.T], 0).astype(f32)
    B_v = np.concatenate([WV_i.T, WV_r.T], 0).astype(f32)

    in_maps = []
    for core in range(NCORES):
        b, hg = core // 2, core % 2
        idx = np.flatnonzero(mask[b])
        cnt = len(idx)

        def cpad(x):  # [Nk, R] -> gathered+padded [nkp, R]
            out = np.zeros((nkp, R), f32)
            out[:cnt] = x[idx]
            return out

        qtf = np.concatenate([Q_real[b].T, Q_imag[b].T], 0)      # [512, NQ]
        ktf = np.concatenate([cpad(K_real[b]).T, cpad(K_imag[b]).T], 0)
        vtf = np.concatenate([cpad(V_real[b]).T, cpad(V_imag[b]).T], 0)

        wq_l = np.empty((NHL, 512, 128), f32)
        wk_l = np.empty((NHL, 512, 128), f32)
        wv_l = np.empty((512, 512), f32)
        wo_l = np.empty((NHL, 128, 512), f32)
        for h in range(NHL):
            g = hg * NHL + h
            gc = slice(g * DK, (g + 1) * DK)
            wq_l[h, :, 0:64] = A_q[:, gc]
            wq_l[h, :, 64:128] = B_q[:, gc]
            wk_l[h, :, 0:64] = A_k[:, gc]
            wk_l[h, :, 64:128] = B_k[:, gc]
            wv_l[:, h * 128:h * 128 + 64] = A_v[:, gc]
            wv_l[:, h * 128 + 64:(h + 1) * 128] = B_v[:, gc]
            woa = np.concatenate([WO_r[:, gc].T, -WO_i[:, gc].T], 0)  # [128, 256]
            wob = np.concatenate([WO_i[:, gc].T, WO_r[:, gc].T], 0)
            wo_l[h, :, 0:256] = woa
            wo_l[h, :, 256:512] = wob

        # device layouts: [128, chunk-major free dim], one DMA per tensor
        qt_dev = _ctile(qtf).astype(_BF16)
        kt_dev = _ctile(ktf).astype(_BF16)
        vt_dev = _ctile(vtf).astype(_BF16)
        wq_dev = np.concatenate([_ctile(wq_l[h]) for h in range(NHL)], 1).astype(_BF16)
        wk_dev = np.concatenate([_ctile(wk_l[h]) for h in range(NHL)], 1).astype(_BF16)
        wv_dev = _ctile(wv_l).astype(_BF16)
        wo_dev = np.concatenate([wo_l[h] for h in range(NHL)], 1).astype(_BF16)

        # padded keys have K == V == 0, so u == +0.0 exactly and the custom
        # activation table maps them to e == 0: no bias tensor needed
        in_maps.append({
            "qt": qt_dev, "kt": kt_dev, "vt": vt_dev,
            "wq": wq_dev, "wk": wk_dev, "wv": wv_dev, "wo": wo_dev,
        })
    return in_maps, nkp, valid


def _gather(results, valid):
    out = np.zeros((B, NQ, R), np.complex64)
    for b in range(B):
        if not valid[b]:
            continue
        r = (results[2 * b]["outr"].astype(np.float32)
             + results[2 * b + 1]["outr"].astype(np.float32))   # [256, NQ]
        i = (results[2 * b]["outi"].astype(np.float32)
             + results[2 * b + 1]["outi"].astype(np.float32))
        out[b] = (r + 1j * i).T
    return out


def _run(inputs, trace=False, trace_kwargs=None):
    from concourse.bass_utils import run_bass_kernel_spmd
    in_maps, nkp, valid = _prep_inputs(**inputs)
    nc = _build(nkp)
    res = run_bass_kernel_spmd(nc, in_maps, core_ids=list(range(NCORES)),
                               trace=trace, **(trace_kwargs or {}))
    return _gather(res.results, valid), res


def kernel(**inputs) -> np.ndarray:
    out, _ = _run(inputs)
    return out
